# revision 42
# baseline (speedup 1.0000x reference)
"""DWA-CNN (DTW-aligned CNN) Trainium2 kernel, v4.

Problem: x[32,2048,128], w[3,128,8], b[8] -> out[32,2046,8]
out[b,p,f] = relu(b[f] + sum of dots along the DTW-optimal path between
window x[b,p:p+3,:] and filter w[:,:,f]).

Math reduction (v3): with all D>=0 the K=3 DP collapses; only 5 path
outcomes exist:
    out = relu(b + d00 + d22 + T),
    [a|bb|ee] = [D11 | D21+min(D10,D11) | D12+min(D01,D11)]
    T = argmin-first([a,bb,ee]):
        a:  d11
        bb: d21 + (D10<=D11 ? d10 : d11)
        ee: d12 + (D01<=D11 ? d01 : d11)
with D(i,j)=Dj[p+i], d(i,j)=qj[p+i] per filter.

v5 schedule changes:
- x loaded as 8 [C,1024] DMAs (128KB DMAs ran at 211GB/s; 256KB at 376).
- NO DRAM bounce at all: every group's psum is reorganized into gm via
  direct engine copies with partition remap (psum[32j+r] -> gm[32g+r]
  @ col j*JW), j=1 on scalar, j=0/j=2 on DVE; halo cols via 3
  shift-by-one stream_shuffles (quadrant row 31 = f7/k3 rows wrap to
  garbage that only feeds discarded outputs). Engine-copy latency
  ~0.1-0.9us vs ~2us per DMA-completion semaphore hop.
- wp loaded via the tensor engine's own queue so the first LDWEIGHTS
  dependency is intra-queue (a cross-queue wait fired ~3us late).
- Select chain: else/true path sums as two paired TTs + one paired CP.
  No gpsimd tensor ops during the chain: gpsimd shares an SBUF port
  with DVE and a concurrent gpsimd TT slowed both engines ~2.4x.
"""
import numpy as np

B, T, C, K, F = 32, 2048, 128, 3, 8
P = T - K + 1            # 2046
NCORES = 8
NB = B // NCORES         # 4 rows per core
TL = NB * T              # 8192 positions per core
FD = 512
NG = 4                   # groups of 4 blocks (group == one batch row)
JW = FD + 2              # 514: block + 2 halo cols
GW = K * JW + 2          # gm width
SCW = 3 * 32 * FD + 8    # scat dram words (pad 8)
NDG = 0                  # groups 0..NDG-1 via DRAM bounce, rest direct
NWARM = 4                # PE warmup loop iterations (2 matmuls each)

# work-tile column offsets (bf16)
EGR = 0                  # Eg/D region, width 3*JW = 1542 (+2 pad)
OU = 1544
OV = OU + 516
OB = OV + 516            # 2576
OE = OB + 516            # 3092
OTM = OE + 516           # 3608 (+4 pad)
OXBE = 4124
OXCE = OXBE + 516
OXBT = OXCE + 516        # 5156
OXCT = OXBT + 516        # 5672
OT = 6188
OACC = 6704
OAC2 = 7220
WTOT = 7736
# mask-tile offsets (u16)
MB, MC, M1, M2 = 0, 516, 1032, 1548
MTOT = 2064

_cache = {}


def _build_program():
    import concourse.tile as tile
    from concourse import bacc, mybir

    f32 = mybir.dt.float32
    bf16 = mybir.dt.bfloat16
    u16 = mybir.dt.uint16
    Alu = mybir.AluOpType
    Act = mybir.ActivationFunctionType

    nc = bacc.Bacc(
        "TRN2",
        target_bir_lowering=False,
        debug=False,
        enable_asserts=False,
        num_devices=NCORES,
    )

    xh = nc.dram_tensor("xh", [C, TL], bf16, kind="ExternalInput").ap()
    wp = nc.dram_tensor("wp", [C, 4 * 96], bf16, kind="ExternalInput").ap()
    nsrj = nc.dram_tensor("nsrj", [C, K * JW], bf16,
                          kind="ExternalInput").ap()
    biasc = nc.dram_tensor("biasc", [C, 2], f32, kind="ExternalInput").ap()
    res = nc.dram_tensor("res", [C, FD], bf16, kind="ExternalOutput").ap()

    from contextlib import ExitStack

    with tile.TileContext(nc) as tc, ExitStack() as ctx:
        const = ctx.enter_context(tc.tile_pool(name="const", bufs=1))
        xin = ctx.enter_context(tc.tile_pool(name="xin", bufs=1))
        psum = ctx.enter_context(tc.tile_pool(name="psum", bufs=1, space="PSUM"))
        stage = ctx.enter_context(tc.tile_pool(name="stage", bufs=1))
        arrs = ctx.enter_context(tc.tile_pool(name="arrs", bufs=1))
        work = ctx.enter_context(tc.tile_pool(name="work", bufs=1))
        dramp = ctx.enter_context(
            tc.tile_pool(name="dramp", bufs=1, space="DRAM"))

        wp_sb = const.tile([C, 4 * 96], bf16, tag="wp")
        nsrj_sb = const.tile([C, K * JW], bf16, tag="nsrj")
        bias_sb = const.tile([C, 2], f32, tag="bias")
        scratch = const.tile([C, 2], f32, tag="scratch")

        # wp/bias/nsrj on the scalar hardware ring (the gpsimd SWDGE ring
        # posts completion semaphores several us late and wp gates the
        # first LDWEIGHTS). x: 4 whole-group tiles [C, 4*FD] -> 4KB
        # row-packets, all on the sync hardware ring, which sustains the
        # best measured per-queue rate. Actual HBM transfers only begin
        # ~3us after the first doorbell (DMA subsystem spin-up).
        nc.scalar.dma_start(wp_sb[:], wp)
        nc.scalar.dma_start(bias_sb[:], biasc)
        # nsrj j=1 slice early on the scalar ring (128KB; gates the first
        # sqrt via Eg1)
        nc.scalar.dma_start(nsrj_sb[:, JW:2 * JW], nsrj[:, JW:2 * JW])
        H = 4 * FD
        xb = [xin.tile([C, H], bf16, tag=f"x{g}", name=f"x{g}")
              for g in range(NG)]
        for g in range(NG):
            nc.sync.dma_start(xb[g][:], xh[:, g * H:(g + 1) * H])
        # nsrj j=0/j=2 slices after all x packets on the sync ring: no
        # contention with the x stream, lands before Egpair needs them
        nsr02d = nsrj[:, 0:JW].unsqueeze(1)
        nsr02d.ap[1] = [2 * JW, 2]
        nsr02s = nsrj_sb[:, 0:JW].unsqueeze(1)
        nsr02s.ap[1] = [2 * JW, 2]
        nc.sync.dma_start(nsr02s, nsr02d)

        # warm the scalar activation tables off the critical path
        nc.scalar.activation(scratch[:, 0:1], bias_sb[:, 0:1], Act.Sqrt)
        nc.scalar.activation(scratch[:, 1:2], bias_sb[:, 0:1], Act.Relu)

        gm = arrs.tile([C, GW], bf16, tag="gm")
        if NDG:
            scat = [dramp.tile([SCW], bf16, tag=f"scat{g}", name=f"scat{g}")
                    for g in range(NDG)]
            stgs = [stage.tile([96, FD], bf16, tag=f"stg{g}", name=f"stg{g}")
                    for g in range(NDG)]

        def scv(g):
            # [p(=f*4+k), j, e(514 over-read)] linear view of scat_g.
            # e=512,513 land on the next block's first cols: the correct
            # halo for k<3; for k=3 rows it's garbage that only feeds
            # windows at positions 2046/2047, which assembly discards.
            v = scat[g][0:3 * 32 * FD].rearrange(
                "(j p e) -> p j e", j=3, p=32)
            v.ap[-1] = [1, JW]
            return v

        def gmv(g):
            return gm[g * 32:(g + 1) * 32, 0:K * JW].rearrange(
                "p (j e) -> p j e", j=K)

        # g-outer matmuls; group g complete after 4 passes -> early copies
        pss = [psum.tile([96, FD], f32, tag=f"ps{g}", name=f"ps{g}")
               for g in range(NG)]
        for g in range(NG):
            for k in range(4):
                nc.tensor.matmul(pss[g][:], wp_sb[:, k * 96:(k + 1) * 96],
                                 xb[g][:, k * FD:(k + 1) * FD],
                                 start=(k == 0), stop=(k == 3))

        # psum->gm copies with partition remap. DVE: j0 all groups + j2 of
        # g0,g1; scalar: j1 all groups + j2 of g2,g3 (so the last group
        # finishes on two engines in parallel). Halo cols via shift-by-one
        # stream_shuffle per j (quadrant row 31 = f7/k3 rows wrap ->
        # garbage that only feeds discarded outputs).
        shmask = [min(r + 1, 31) for r in range(32)]

        def shuffle_j(j):
            nc.vector.stream_shuffle(
                gm[0:128, j * JW + FD:j * JW + FD + 2],
                gm[0:128, j * JW:j * JW + 2], shmask)

        # psum->gm copies. The scalar engine's psum-ready semaphore arrives
        # ~1.5us later than the DVE's, so the last group's j0+j1 go on DVE
        # (j1 gates Eg1 -> first sqrt); j2 of the last group on scalar.
        for g in range(NG):
            r0 = 32 * g
            last = g == NG - 1
            nc.vector.tensor_copy(gm[r0:r0 + 32, 0:FD], pss[g][0:32, :])
            if last:
                nc.vector.tensor_copy(gm[r0:r0 + 32, JW:JW + FD],
                                      pss[g][32:64, :])
            else:
                nc.scalar.copy(gm[r0:r0 + 32, JW:JW + FD], pss[g][32:64, :])
            nc.scalar.copy(gm[r0:r0 + 32, 2 * JW:2 * JW + FD],
                           pss[g][64:96, :])
        shuffle_j(0)
        shuffle_j(1)

        # ---- reduced DTW chain ----
        Wt = work.tile([C, WTOT], bf16, tag="W")
        Mk = work.tile([C, MTOT], u16, tag="M")
        resl = work.tile([C, FD], bf16, tag="res")

        V = nc.vector
        S = nc.scalar
        G = nc.gpsimd
        TT = V.tensor_tensor
        CP = V.copy_predicated

        def win2(ap2d, off_a, off_b, n=FD):
            v = ap2d[:, off_a:off_a + n].unsqueeze(1)
            v.ap[1] = [off_b - off_a, 2]
            return v

        def w2(a, b, n=FD):
            return win2(Wt[:], a, b, n)

        def g2(a, b, n=FD):
            return win2(gm[:], a, b, n)

        def m2(a, b):
            return win2(Mk[:], a, b)

        aO = EGR + JW + 1        # D11 array (a)
        D0O = EGR + 1            # D10 array (j=0 @ +1)
        D1O = EGR + JW           # D1 array @ 0
        D2O = EGR + 2 * JW + 1   # D12 array (j=2 @ +1)

        # Eg = gm + nsrj, j=1 slice first so sqrt(D1) starts while the
        # last j2 copy + j=0/j=2 Eg pair are still on the DVE
        TT(Wt[:, D1O:D1O + JW], gm[:, JW:JW + JW], nsrj_sb[:, JW:JW + JW],
           Alu.add)
        shuffle_j(2)
        TT(win2(Wt[:], 0, 2 * JW, JW), win2(gm[:], 0, 2 * JW, JW),
           win2(nsrj_sb[:], 0, 2 * JW, JW), Alu.add)
        S.activation(Wt[:, D1O:D1O + JW], Wt[:, D1O:D1O + JW], Act.Sqrt)
        S.activation(w2(D0O, D2O), w2(D0O, D2O), Act.Sqrt)
        # during the scalar sqrts, DVE computes gm-only leaves:
        # else-path sums [XBe|XCe] = [d21|d12] + d11, and ACCM = d00+d22
        TT(w2(OXBE, OXCE), g2(JW + 2, 2 * JW + 1), g2(JW + 1, JW + 1),
           Alu.add)
        TT(Wt[:, OACC:OACC + FD], gm[:, 0:FD],
           gm[:, 2 * JW + 2:2 * JW + 2 + FD], Alu.add)
        # true-path sums: [XBt|XCt] = [d21|d12] + [d10|d01]
        # (on DVE: a concurrent gpsimd TT contends for the shared SBUF
        # port and slows both engines ~2.4x)
        TT(w2(OXBT, OXCT), g2(JW + 2, 2 * JW + 1), g2(1, JW), Alu.add)
        # [u|v] = min([D10|D01], [a|a])
        TT(w2(OU, OV), w2(D0O, D1O), w2(aO, aO), Alu.min)
        # [mB|mC] = is_le([D10|D01], [a|a])
        TT(m2(MB, MC), w2(D0O, D1O), w2(aO, aO), Alu.is_le)
        # [bb|ee] = [D21|D12] + [u|v]
        TT(w2(OB, OE), w2(D1O + 2, D2O), w2(OU, OV), Alu.add)
        # t_mn = min(a, bb)
        TT(Wt[:, OTM:OTM + FD], Wt[:, aO:aO + FD], Wt[:, OB:OB + FD],
           Alu.min)
        # [m1|m2] = is_lt([bb|ee], [a|t_mn])
        TT(m2(M1, M2), w2(OB, OE), w2(aO, OTM), Alu.is_lt)
        # inner selects: [XBe|XCe] <- [XBt|XCt] where [mB|mC]
        CP(w2(OXBE, OXCE), m2(MB, MC), w2(OXBT, OXCT))
        # T cascade IN-PLACE on gm1@1 (= T default d11; gm1@1 has no
        # readers after the XBe/XBt sums): T <- XB where m1 (bb<a);
        # T <- XC where m2 (ee<min). Saves a 512-col copy.
        tsl = gm[:, JW + 1:JW + 1 + FD]
        CP(tsl, Mk[:, M1:M1 + FD], Wt[:, OXBE:OXBE + FD])
        CP(tsl, Mk[:, M2:M2 + FD], Wt[:, OXCE:OXCE + FD])
        # ACC2 = ACCM + T ; out = relu(-0.5*ACC2 + b)
        TT(Wt[:, OAC2:OAC2 + FD], Wt[:, OACC:OACC + FD], tsl, Alu.add)
        Hf = FD // 2
        for h in range(2):
            sl = slice(h * Hf, (h + 1) * Hf)
            S.activation(resl[:, sl], Wt[:, OAC2 + h * Hf:OAC2 + (h + 1) * Hf],
                         Act.Relu, bias=bias_sb[:, 0:1], scale=-0.5)
            # both halves on the sync ring: the scalar ring's packet queue
            # is slow and trails ~1.5us past the last compute
            nc.sync.dma_start(res[:, sl], resl[:, sl])

    nc.compile()
    return nc


def _host_prep(x, w, b):
    import ml_dtypes

    x = np.ascontiguousarray(np.asarray(x, np.float32))
    w = np.asarray(w, np.float32)
    b = np.asarray(b, np.float32)

    # stationary: wp[:, k*96 + (j*32 + f*4 + kslot)] = -2w[j,:,f] iff kslot==k
    wp = np.zeros((C, 4 * 96), np.float32)
    for k in range(4):
        for j in range(K):
            for f in range(F):
                wp[:, k * 96 + j * 32 + f * 4 + k] = -2.0 * w[j, :, f]
    wp = wp.astype(ml_dtypes.bfloat16)

    nW = (w ** 2).sum(1)                                  # [K, F]
    fmap = (np.arange(C) % 32) // 4
    biasc = np.zeros((C, 2), np.float32)
    biasc[:, 0] = b[fmap]

    in_maps = []
    for r in range(NCORES):
        x4 = x[r * NB:(r + 1) * NB]                       # [NB,T,C]
        flat = x4.reshape(TL, C)
        xT = np.ascontiguousarray(flat.T)                 # [C, TL] fp32
        xhh = xT.astype(ml_dtypes.bfloat16)
        nS = np.einsum("tc,tc->t", flat, flat).astype(np.float32)
        nsp = np.ones((C, K * JW), np.float32)
        for p in range(C):
            g = p // 32
            k = p % 4
            f = fmap[p]
            t0 = (4 * g + k) * FD
            hi = min(TL, t0 + JW)
            n = hi - t0
            for j in range(K):
                nsp[p, j * JW:j * JW + n] = nS[t0:hi] + nW[j, f]
                if n < JW:
                    nsp[p, j * JW + n:(j + 1) * JW] = nW[j, f] + 1.0
        in_maps.append({
            "xh": xhh, "wp": wp, "nsrj": nsp.astype(ml_dtypes.bfloat16),
            "biasc": biasc,
        })
    return in_maps


def _assemble(results):
    out = np.empty((B, P, F), np.float32)
    for r in range(NCORES):
        resr = np.asarray(results[r]["res"], np.float32)  # [128, 512]
        arr = resr.reshape(4, 8, 4, FD)                   # [g, f, k, e]
        # out[r*NB+g, k*512+e, f] = arr[g, f, k, e]
        series = arr.transpose(0, 2, 3, 1).reshape(4, T, F)  # [g, pos, f]
        out[r * NB:(r + 1) * NB] = series[:, :P, :]
    return out


def kernel(x, w, b):
    from concourse.bass_utils import run_bass_kernel_spmd

    if "nc" not in _cache:
        _cache["nc"] = _build_program()
    nc = _cache["nc"]
    in_maps = _host_prep(x, w, b)
    out = run_bass_kernel_spmd(nc, in_maps, core_ids=list(range(NCORES)))
    return _assemble(out.results)


if __name__ == "__main__":
    rng = np.random.default_rng(0)
    x = rng.standard_normal((B, T, C), dtype=np.float32)
    w = (rng.standard_normal((K, C, F)) * 0.1).astype(np.float32)
    b = np.zeros((F,), np.float32)
    o = kernel(x, w, b)
    print("kernel ran, out shape", o.shape, float(np.abs(o).sum()))


# revision 43
# speedup vs baseline: 1.0424x; 1.0424x over previous
"""DWA-CNN (DTW-aligned CNN) Trainium2 kernel, v12.

Problem: x[32,2048,128], w[3,128,8], b[8] -> out[32,2046,8]
out[b,p,f] = relu(b[f] + sum of dots along the DTW-optimal path between
window x[b,p:p+3,:] and filter w[:,:,f]).

Math reduction: with all D>=0 the K=3 DTW DP collapses (c21=D10+c11,
c12=D01+c11, c22=D11+c11 always; backtrack from (2,2) is always
diagonal), so only 5 path outcomes exist:
    out = relu(b + d00 + d22 + T),
    [a|bb|ee] = [D11 | D21+min(D10,D11) | D12+min(D01,D11)]
    T = argmin-first([a,bb,ee]):
        a:  d11
        bb: d21 + (D10<=D11 ? d10 : d11)
        ee: d12 + (D01<=D11 ? d01 : d11)
with D(i,j)=Dj[p+i], d(i,j)=qj[p+i] per filter. ~14 DVE ops vs ~29 for
the full DP/backtrack.

Schedule (8 cores, data parallel over batch, 4 rows/core, TL=8192):
- bf16 matmul, stationary packs -2*w[j,:,f] into psum row j*32+f*4+k
  (kslot-masked, 4 accumulating passes per group of four 512-blocks).
- x: 4 whole-group [C,2048] DMAs on the sync hardware ring (4KB row
  packets); nsrj j1-slice early on the scalar ring (gates first sqrt),
  j0/j2 behind x on the sync ring. Real HBM transfers begin ~3us after
  the first doorbell (DMA spin-up) and x arrival paces the matmuls.
- NO DRAM bounce: psum reorganized into gm via engine copies with
  partition remap (psum[32j+r] -> gm[32g+r] @ col j*JW); halo cols via
  shift-by-one stream_shuffles (quadrant row 31 = f7/k3 rows wrap to
  garbage that only feeds discarded outputs). The last group's j0+j1
  copies ride the DVE (its psum-ready semaphore arrives ~1.5us before
  the scalar engine's); Eg1/sqrt start while j2 still copies.
- Select chain: else/true path sums as paired TTs + one paired CP; the
  T cascade CPs in-place on gm1@1. No gpsimd tensor ops anywhere near
  the chain: gpsimd shares an SBUF port with the DVE and a concurrent
  gpsimd TT slowed both engines ~2.4x.
- Output bf16 (the pre-relu value is already bf16 -> lossless), both
  halves on the sync ring (the scalar ring's packet queue trails ~1.5us
  past the last compute).
"""
import numpy as np

B, T, C, K, F = 32, 2048, 128, 3, 8
P = T - K + 1            # 2046
NCORES = 8
NB = B // NCORES         # 4 rows per core
TL = NB * T              # 8192 positions per core
FD = 512
NG = 4                   # groups of 4 blocks (group == one batch row)
JW = FD + 2              # 514: block + 2 halo cols
GW = K * JW + 2          # gm width
SCW = 3 * 32 * FD + 8    # scat dram words (pad 8)
NDG = 0                  # groups 0..NDG-1 via DRAM bounce, rest direct
NWARM = 4                # PE warmup loop iterations (2 matmuls each)

# work-tile column offsets (bf16)
EGR = 0                  # Eg/D region, width 3*JW = 1542 (+2 pad)
OU = 1544
OV = OU + 516
OB = OV + 516            # 2576
OE = OB + 516            # 3092
OTM = OE + 516           # 3608 (+4 pad)
OXBE = 4124
OXCE = OXBE + 516
OXBT = OXCE + 516        # 5156
OXCT = OXBT + 516        # 5672
OT = 6188
OACC = 6704
OAC2 = 7220
WTOT = 7736
# mask-tile offsets (u16)
MB, MC, M1, M2 = 0, 516, 1032, 1548
MTOT = 2064

_cache = {}


def _build_program():
    import concourse.tile as tile
    from concourse import bacc, mybir

    f32 = mybir.dt.float32
    bf16 = mybir.dt.bfloat16
    u16 = mybir.dt.uint16
    Alu = mybir.AluOpType
    Act = mybir.ActivationFunctionType

    nc = bacc.Bacc(
        "TRN2",
        target_bir_lowering=False,
        debug=False,
        enable_asserts=False,
        num_devices=NCORES,
    )

    xh = nc.dram_tensor("xh", [C, TL], bf16, kind="ExternalInput").ap()
    wp = nc.dram_tensor("wp", [C, 4 * 96], bf16, kind="ExternalInput").ap()
    nsrj = nc.dram_tensor("nsrj", [C, K * JW], bf16,
                          kind="ExternalInput").ap()
    biasc = nc.dram_tensor("biasc", [C, 2], f32, kind="ExternalInput").ap()
    res = nc.dram_tensor("res", [C, FD], bf16, kind="ExternalOutput").ap()

    from contextlib import ExitStack

    with tile.TileContext(nc) as tc, ExitStack() as ctx:
        const = ctx.enter_context(tc.tile_pool(name="const", bufs=1))
        xin = ctx.enter_context(tc.tile_pool(name="xin", bufs=1))
        psum = ctx.enter_context(tc.tile_pool(name="psum", bufs=1, space="PSUM"))
        stage = ctx.enter_context(tc.tile_pool(name="stage", bufs=1))
        arrs = ctx.enter_context(tc.tile_pool(name="arrs", bufs=1))
        work = ctx.enter_context(tc.tile_pool(name="work", bufs=1))
        dramp = ctx.enter_context(
            tc.tile_pool(name="dramp", bufs=1, space="DRAM"))

        wp_sb = const.tile([C, 4 * 96], bf16, tag="wp")
        nsrj_sb = const.tile([C, K * JW], bf16, tag="nsrj")
        bias_sb = const.tile([C, 2], f32, tag="bias")
        scratch = const.tile([C, 2], f32, tag="scratch")

        # wp/bias/nsrj on the scalar hardware ring (the gpsimd SWDGE ring
        # posts completion semaphores several us late and wp gates the
        # first LDWEIGHTS). x: 4 whole-group tiles [C, 4*FD] -> 4KB
        # row-packets, all on the sync hardware ring, which sustains the
        # best measured per-queue rate. Actual HBM transfers only begin
        # ~3us after the first doorbell (DMA subsystem spin-up).
        nc.scalar.dma_start(wp_sb[:], wp)
        nc.scalar.dma_start(bias_sb[:], biasc)
        # nsrj j=1 slice early on the scalar ring (128KB; gates the first
        # sqrt via Eg1)
        nc.scalar.dma_start(nsrj_sb[:, JW:2 * JW], nsrj[:, JW:2 * JW])
        H = 4 * FD
        xb = [xin.tile([C, H], bf16, tag=f"x{g}", name=f"x{g}")
              for g in range(NG)]
        for g in range(NG):
            nc.sync.dma_start(xb[g][:], xh[:, g * H:(g + 1) * H])
        # nsrj j=0/j=2 slices after all x packets on the sync ring: no
        # contention with the x stream, lands before Egpair needs them
        nsr02d = nsrj[:, 0:JW].unsqueeze(1)
        nsr02d.ap[1] = [2 * JW, 2]
        nsr02s = nsrj_sb[:, 0:JW].unsqueeze(1)
        nsr02s.ap[1] = [2 * JW, 2]
        nc.sync.dma_start(nsr02s, nsr02d)

        # warm the scalar activation tables off the critical path
        nc.scalar.activation(scratch[:, 0:1], bias_sb[:, 0:1], Act.Sqrt)
        nc.scalar.activation(scratch[:, 1:2], bias_sb[:, 0:1], Act.Relu)

        gm = arrs.tile([C, GW], bf16, tag="gm")
        if NDG:
            scat = [dramp.tile([SCW], bf16, tag=f"scat{g}", name=f"scat{g}")
                    for g in range(NDG)]
            stgs = [stage.tile([96, FD], bf16, tag=f"stg{g}", name=f"stg{g}")
                    for g in range(NDG)]

        def scv(g):
            # [p(=f*4+k), j, e(514 over-read)] linear view of scat_g.
            # e=512,513 land on the next block's first cols: the correct
            # halo for k<3; for k=3 rows it's garbage that only feeds
            # windows at positions 2046/2047, which assembly discards.
            v = scat[g][0:3 * 32 * FD].rearrange(
                "(j p e) -> p j e", j=3, p=32)
            v.ap[-1] = [1, JW]
            return v

        def gmv(g):
            return gm[g * 32:(g + 1) * 32, 0:K * JW].rearrange(
                "p (j e) -> p j e", j=K)

        # g-outer matmuls; group g complete after 4 passes -> early copies
        pss = [psum.tile([96, FD], f32, tag=f"ps{g}", name=f"ps{g}")
               for g in range(NG)]
        for g in range(NG):
            for k in range(4):
                nc.tensor.matmul(pss[g][:], wp_sb[:, k * 96:(k + 1) * 96],
                                 xb[g][:, k * FD:(k + 1) * FD],
                                 start=(k == 0), stop=(k == 3))

        # psum->gm copies with partition remap. DVE: j0 all groups + j2 of
        # g0,g1; scalar: j1 all groups + j2 of g2,g3 (so the last group
        # finishes on two engines in parallel). Halo cols via shift-by-one
        # stream_shuffle per j (quadrant row 31 = f7/k3 rows wrap ->
        # garbage that only feeds discarded outputs).
        shmask = [min(r + 1, 31) for r in range(32)]

        def shuffle_j(j):
            nc.vector.stream_shuffle(
                gm[0:128, j * JW + FD:j * JW + FD + 2],
                gm[0:128, j * JW:j * JW + 2], shmask)

        # psum->gm copies. The scalar engine's psum-ready semaphore arrives
        # ~1.5us later than the DVE's, so the last group's j0+j1 go on DVE
        # (j1 gates Eg1 -> first sqrt); j2 of the last group on scalar.
        for g in range(NG):
            r0 = 32 * g
            last = g == NG - 1
            nc.vector.tensor_copy(gm[r0:r0 + 32, 0:FD], pss[g][0:32, :])
            if last:
                nc.vector.tensor_copy(gm[r0:r0 + 32, JW:JW + FD],
                                      pss[g][32:64, :])
            else:
                nc.scalar.copy(gm[r0:r0 + 32, JW:JW + FD], pss[g][32:64, :])
            nc.scalar.copy(gm[r0:r0 + 32, 2 * JW:2 * JW + FD],
                           pss[g][64:96, :])
        shuffle_j(0)
        shuffle_j(1)

        # ---- reduced DTW chain ----
        Wt = work.tile([C, WTOT], bf16, tag="W")
        Mk = work.tile([C, MTOT], u16, tag="M")
        resl = work.tile([C, FD], bf16, tag="res")

        V = nc.vector
        S = nc.scalar
        G = nc.gpsimd
        TT = V.tensor_tensor
        CP = V.copy_predicated

        def win2(ap2d, off_a, off_b, n=FD):
            v = ap2d[:, off_a:off_a + n].unsqueeze(1)
            v.ap[1] = [off_b - off_a, 2]
            return v

        def w2(a, b, n=FD):
            return win2(Wt[:], a, b, n)

        def g2(a, b, n=FD):
            return win2(gm[:], a, b, n)

        def m2(a, b):
            return win2(Mk[:], a, b)

        aO = EGR + JW + 1        # D11 array (a)
        D0O = EGR + 1            # D10 array (j=0 @ +1)
        D1O = EGR + JW           # D1 array @ 0
        D2O = EGR + 2 * JW + 1   # D12 array (j=2 @ +1)

        # Eg = gm + nsrj, j=1 slice first so sqrt(D1) starts while the
        # last j2 copy + j=0/j=2 Eg pair are still on the DVE
        TT(Wt[:, D1O:D1O + JW], gm[:, JW:JW + JW], nsrj_sb[:, JW:JW + JW],
           Alu.add)
        shuffle_j(2)
        TT(win2(Wt[:], 0, 2 * JW, JW), win2(gm[:], 0, 2 * JW, JW),
           win2(nsrj_sb[:], 0, 2 * JW, JW), Alu.add)
        S.activation(Wt[:, D1O:D1O + JW], Wt[:, D1O:D1O + JW], Act.Sqrt)
        S.activation(w2(D0O, D2O), w2(D0O, D2O), Act.Sqrt)
        # during the scalar sqrts, DVE computes gm-only leaves:
        # else-path sums [XBe|XCe] = [d21|d12] + d11, and ACCM = d00+d22
        TT(w2(OXBE, OXCE), g2(JW + 2, 2 * JW + 1), g2(JW + 1, JW + 1),
           Alu.add)
        TT(Wt[:, OACC:OACC + FD], gm[:, 0:FD],
           gm[:, 2 * JW + 2:2 * JW + 2 + FD], Alu.add)
        # true-path sums: [XBt|XCt] = [d21|d12] + [d10|d01]
        # (on DVE: a concurrent gpsimd TT contends for the shared SBUF
        # port and slows both engines ~2.4x)
        TT(w2(OXBT, OXCT), g2(JW + 2, 2 * JW + 1), g2(1, JW), Alu.add)
        # [u|v] = min([D10|D01], [a|a])
        TT(w2(OU, OV), w2(D0O, D1O), w2(aO, aO), Alu.min)
        # [mB|mC] = is_le([D10|D01], [a|a])
        TT(m2(MB, MC), w2(D0O, D1O), w2(aO, aO), Alu.is_le)
        # [bb|ee] = [D21|D12] + [u|v]
        TT(w2(OB, OE), w2(D1O + 2, D2O), w2(OU, OV), Alu.add)
        # t_mn = min(a, bb)
        TT(Wt[:, OTM:OTM + FD], Wt[:, aO:aO + FD], Wt[:, OB:OB + FD],
           Alu.min)
        # [m1|m2] = is_lt([bb|ee], [a|t_mn])
        TT(m2(M1, M2), w2(OB, OE), w2(aO, OTM), Alu.is_lt)
        # inner selects: [XBe|XCe] <- [XBt|XCt] where [mB|mC]
        CP(w2(OXBE, OXCE), m2(MB, MC), w2(OXBT, OXCT))
        # T cascade IN-PLACE on gm1@1 (= T default d11; gm1@1 has no
        # readers after the XBe/XBt sums): T <- XB where m1 (bb<a);
        # T <- XC where m2 (ee<min). Saves a 512-col copy.
        tsl = gm[:, JW + 1:JW + 1 + FD]
        CP(tsl, Mk[:, M1:M1 + FD], Wt[:, OXBE:OXBE + FD])
        CP(tsl, Mk[:, M2:M2 + FD], Wt[:, OXCE:OXCE + FD])
        # ACC2 = ACCM + T ; out = relu(-0.5*ACC2 + b)
        TT(Wt[:, OAC2:OAC2 + FD], Wt[:, OACC:OACC + FD], tsl, Alu.add)
        Hf = FD // 2
        for h in range(2):
            sl = slice(h * Hf, (h + 1) * Hf)
            S.activation(resl[:, sl], Wt[:, OAC2 + h * Hf:OAC2 + (h + 1) * Hf],
                         Act.Relu, bias=bias_sb[:, 0:1], scale=-0.5)
            # both halves on the sync ring: the scalar ring's packet queue
            # is slow and trails ~1.5us past the last compute
            nc.sync.dma_start(res[:, sl], resl[:, sl])

    nc.compile()
    return nc


def _host_prep(x, w, b):
    import ml_dtypes

    x = np.ascontiguousarray(np.asarray(x, np.float32))
    w = np.asarray(w, np.float32)
    b = np.asarray(b, np.float32)

    # stationary: wp[:, k*96 + (j*32 + f*4 + kslot)] = -2w[j,:,f] iff kslot==k
    wp = np.zeros((C, 4 * 96), np.float32)
    for k in range(4):
        for j in range(K):
            for f in range(F):
                wp[:, k * 96 + j * 32 + f * 4 + k] = -2.0 * w[j, :, f]
    wp = wp.astype(ml_dtypes.bfloat16)

    nW = (w ** 2).sum(1)                                  # [K, F]
    fmap = (np.arange(C) % 32) // 4
    biasc = np.zeros((C, 2), np.float32)
    biasc[:, 0] = b[fmap]

    in_maps = []
    for r in range(NCORES):
        x4 = x[r * NB:(r + 1) * NB]                       # [NB,T,C]
        flat = x4.reshape(TL, C)
        xT = np.ascontiguousarray(flat.T)                 # [C, TL] fp32
        xhh = xT.astype(ml_dtypes.bfloat16)
        nS = np.einsum("tc,tc->t", flat, flat).astype(np.float32)
        nsp = np.ones((C, K * JW), np.float32)
        for p in range(C):
            g = p // 32
            k = p % 4
            f = fmap[p]
            t0 = (4 * g + k) * FD
            hi = min(TL, t0 + JW)
            n = hi - t0
            for j in range(K):
                nsp[p, j * JW:j * JW + n] = nS[t0:hi] + nW[j, f]
                if n < JW:
                    nsp[p, j * JW + n:(j + 1) * JW] = nW[j, f] + 1.0
        in_maps.append({
            "xh": xhh, "wp": wp, "nsrj": nsp.astype(ml_dtypes.bfloat16),
            "biasc": biasc,
        })
    return in_maps


def _assemble(results):
    out = np.empty((B, P, F), np.float32)
    for r in range(NCORES):
        resr = np.asarray(results[r]["res"], np.float32)  # [128, 512]
        arr = resr.reshape(4, 8, 4, FD)                   # [g, f, k, e]
        # out[r*NB+g, k*512+e, f] = arr[g, f, k, e]
        series = arr.transpose(0, 2, 3, 1).reshape(4, T, F)  # [g, pos, f]
        out[r * NB:(r + 1) * NB] = series[:, :P, :]
    return out


def kernel(x, w, b):
    from concourse.bass_utils import run_bass_kernel_spmd

    if "nc" not in _cache:
        _cache["nc"] = _build_program()
    nc = _cache["nc"]
    in_maps = _host_prep(x, w, b)
    out = run_bass_kernel_spmd(nc, in_maps, core_ids=list(range(NCORES)))
    return _assemble(out.results)


if __name__ == "__main__":
    rng = np.random.default_rng(0)
    x = rng.standard_normal((B, T, C), dtype=np.float32)
    w = (rng.standard_normal((K, C, F)) * 0.1).astype(np.float32)
    b = np.zeros((F,), np.float32)
    o = kernel(x, w, b)
    print("kernel ran, out shape", o.shape, float(np.abs(o).sum()))
